# revision 2
# baseline (speedup 1.0000x reference)
"""Trainium2 Bass kernel v2 for the per-batch attention block.

Reference math (per batch b, with C=E=512, H=W=32 -> N=1024, heads=8, d=64):
    qkv = w_in @ x_flat                      # [3E, N]
    S_h = q_h^T k_h * heads**-0.5            # [N, N] per head
    P_h = softmax(S_h, axis=-1)
    o_h = v_h @ P_h^T                        # [d, N]
    out = w_out @ concat(o_h) + b_out + x_flat

Data-parallel over batch across 8 NeuronCores (one batch element/core).

v2 layout changes vs v1:
  - O matmuls produce o^T (positions on partitions): per (head, i-chunk)
    psum [128, 65] accumulated over 8 k-chunks; col 0 is the softmax
    denominator (ones-column trick), so normalization is a per-partition
    reciprocal + tensor_scalar fused into the PSUM evacuation. This kills
    the DRAM-bounce broadcast / gpsimd multiply of v1 and halves the O
    matmul cycles (full 128-row PE utilization).
  - o^T -> o via PE transposes ([128,128] blocks against an identity),
    evacuated to SBUF for the output projection.
  - bias folded into the residual input (xb = x + b) once outside the
    reps loop; no K=1 bias matmuls.
  - software-pipelined schedule: region j emits S/exp(pair j) on PE/ACT
    while interleaving O+normalize(pair j-1), q/k projections for pair
    j+1, and transposes, keeping ACT (the exp wall, ~66us) saturated.
"""

import sys

if "/opt/trn_rl_repo" not in sys.path:
    sys.path.insert(0, "/opt/trn_rl_repo")

from contextlib import ExitStack, nullcontext

import numpy as np

import concourse.bass as bass
import concourse.tile as tile
from concourse import bacc, mybir
from concourse.bass_utils import run_bass_kernel_spmd
from concourse.masks import make_identity

F32 = mybir.dt.float32
F32R = mybir.dt.float32r
F16 = mybir.dt.float16
ESHIFT = -10.0  # exp bias shift: keeps P = exp(S*scale-10) within fp16 range
EXP = mybir.ActivationFunctionType.Exp

C = 512
N = 1024
E = 512
HEADS = 8
D = 64
NH = D + 1  # ones column + 64 v-channels per head
SCALE = float(HEADS) ** -0.5
P = 128
N_CORES = 8


def _build(n_cores=N_CORES, reps=1):
    nc = bacc.Bacc(
        "TRN2", target_bir_lowering=False, debug=False, num_devices=n_cores
    )
    x_d = nc.dram_tensor("x", [C, N], F32R, kind="ExternalInput").ap()
    wqkT_d = nc.dram_tensor("wqkT", [C, 2 * E], F32R, kind="ExternalInput").ap()
    wvT_d = nc.dram_tensor("wvT", [C, E], F32R, kind="ExternalInput").ap()
    woutT_d = nc.dram_tensor("woutT", [E, C], F32R, kind="ExternalInput").ap()
    # bias pre-shaped [128, 4] host-side: col c = b_out[c*128:(c+1)*128]
    bias_d = nc.dram_tensor("bias", [P, 4], F32, kind="ExternalInput").ap()
    out_d = nc.dram_tensor("out", [C, N], F32, kind="ExternalOutput").ap()

    with tile.TileContext(nc) as tc, ExitStack() as ctx:
        consts = ctx.enter_context(tc.tile_pool(name="consts", bufs=1))
        qk_pool = ctx.enter_context(tc.tile_pool(name="qk", bufs=1))
        vt_pool = ctx.enter_context(tc.tile_pool(name="vt", bufs=1))
        osb_pool = ctx.enter_context(tc.tile_pool(name="osb", bufs=1))
        misc_pool = ctx.enter_context(tc.tile_pool(name="misc", bufs=2))

        # ---- load inputs (issue order = first-use order) ----------------
        xf = []
        wqkT = []
        wvT = []
        woutT = []
        for c in range(4):
            tx = consts.tile([P, N], F32R, tag=f"xf{c}", name=f"xf{c}")
            nc.sync.dma_start(tx[:], x_d[c * P : (c + 1) * P, :])
            xf.append(tx)
            tw = consts.tile([P, 2 * E], F32R, tag=f"wqkT{c}", name=f"wqkT{c}")
            nc.scalar.dma_start(tw[:], wqkT_d[c * P : (c + 1) * P, :])
            wqkT.append(tw)
        for c in range(4):
            t = consts.tile([P, E], F32R, tag=f"wvT{c}", name=f"wvT{c}")
            nc.sync.dma_start(t[:], wvT_d[c * P : (c + 1) * P, :])
            wvT.append(t)
        for e in range(4):
            t = consts.tile([P, C], F32R, tag=f"woutT{e}", name=f"woutT{e}")
            nc.scalar.dma_start(t[:], woutT_d[e * P : (e + 1) * P, :])
            woutT.append(t)
        # bias as a [128, 4] column tile: col c = b_out[c*128:(c+1)*128]
        bias_sb = consts.tile([P, 4], F32, tag="bias", name="bias_sb")
        nc.scalar.dma_start(bias_sb[:], bias_d[:])
        eshift_sb = consts.tile([P, 1], F32, tag="eshift", name="eshift_sb")
        nc.vector.memset(eshift_sb[:], ESHIFT)
        ident = consts.tile([P, P], F16, tag="ident", name="ident")
        make_identity(nc, ident[:])
        # residual-with-bias input, computed once
        xb = []
        for c in range(4):
            t = consts.tile([P, N], F32, tag=f"xb{c}", name=f"xb{c}")
            nc.vector.tensor_scalar_add(
                t[:], xf[c][:].bitcast(F32), bias_sb[:, c : c + 1]
            )
            xb.append(t)

        # persistent attention SBUF tiles
        qk_sb = [None] * 8
        vt_sb = []
        for n in range(8):
            t = vt_pool.tile([P, HEADS * NH], F16, tag=f"vt{n}", name=f"vt{n}")
            vt3 = t[:].rearrange("p (h d) -> p h d", h=HEADS)
            nc.vector.memset(vt3[:, :, 0:1], 1.0)  # ones cols, never rewritten
            vt_sb.append(t)
        osb = []
        for j in range(4):
            t = osb_pool.tile([P, N], F32R, tag=f"osb{j}", name=f"osb{j}")
            osb.append(t)

        rep_ctx = (
            tc.For_i(0, reps, 1, hint_engines=(mybir.EngineType.PE,))
            if reps > 1
            else nullcontext()
        )
        with (
            tc.tile_pool(name="ps", bufs=1, space="PSUM") as ps,
            tc.tile_pool(name="pt", bufs=2) as pt_pool,
            tc.tile_pool(name="norm", bufs=2) as norm_pool,
            rep_ctx,
        ):
            # PSUM budget (8 banks): s0,s1 = [128,1024] (2 banks each) for
            # the two heads' S^T tiles; oc0,oc1 = 1 bank each (C psums +
            # o^T chunk accumulators); tp = [128,1024] (2 banks) shared by
            # q/k projection psums, transposes, and an E-phase wave.

            b_psums = {}

            def emit_B_half(m, half, tag="tp"):
                # half 0: c=0,1 (start), half 1: c=2,3 (stop) + evacuation
                if half == 0:
                    b_psums[m] = ps.tile([P, N], F32, tag=tag, name=f"psB{m}")
                psum = b_psums.pop(m) if half == 1 else b_psums[m]
                for c in (0, 1) if half == 0 else (2, 3):
                    for ih in range(2):
                        nc.tensor.matmul(
                            psum[:, ih * 512 : (ih + 1) * 512],
                            wqkT[c][:, m * P : (m + 1) * P],
                            xf[c][:, ih * 512 : (ih + 1) * 512],
                            start=(c == 0),
                            stop=(c == 3),
                        )
                if half == 1:
                    t = qk_pool.tile([P, N], F16, tag=f"qk{m}", name=f"qk{m}")
                    nc.vector.tensor_copy(t[:], psum[:])
                    qk_sb[m] = t

            def emit_C(n):
                psum = ps.tile([P, E], F32, tag=f"oc{n % 2}", name=f"psC{n}")
                for c in range(4):
                    nc.tensor.matmul(
                        psum[:],
                        xf[c][:, n * P : (n + 1) * P],
                        wvT[c][:],
                        start=(c == 0),
                        stop=(c == 3),
                    )
                t3 = vt_sb[n][:].rearrange("p (h d) -> p h d", h=HEADS)
                nc.vector.tensor_copy(
                    t3[:, :, 1:NH], psum[:].rearrange("p (h d) -> p h d", h=HEADS)
                )

            def emit_O_chunk(jprev, pts, onT, c):
                # o^T accumulation for i-chunk c of pair jprev, both heads:
                # psum [128 i, 65] over 8 k-chunks; col 0 = denominator.
                for hslot in range(2):
                    h = 2 * jprev + hslot
                    o_ps = ps.tile(
                        [P, NH], F32, tag=f"oc{(c + hslot) % 2}", name=f"oT{h}_{c}"
                    )
                    for a in range(8):
                        nc.tensor.matmul(
                            o_ps[:],
                            pts[hslot][a][:, c * P : (c + 1) * P],
                            vt_sb[a][:, h * NH : (h + 1) * NH],
                            start=(a == 0),
                            stop=(a == 7),
                        )
                    r = norm_pool.tile([P, 1], F32, tag=f"r{hslot}", name=f"r{h}_{c}")
                    nc.vector.reciprocal(r[:], o_ps[:, 0:1])
                    nc.vector.tensor_scalar_mul(
                        onT[:, c * P + hslot * D : c * P + (hslot + 1) * D],
                        o_ps[:, 1:NH],
                        r[:],
                    )

            def emit_transposes(jprev, onT):
                tp_ps = ps.tile([P, N], F16, tag="tp", name=f"tp{jprev}")
                for c in range(8):
                    nc.tensor.transpose(
                        tp_ps[:, c * P : (c + 1) * P],
                        onT[:, c * P : (c + 1) * P],
                        ident[:],
                    )
                nc.vector.tensor_copy(osb[jprev][:], tp_ps[:])

            def emit_region(j, prev_ctx):
                # S/exp for pair j; filler work interleaved per a-step.
                ptA = [None] * 8
                ptB = [None] * 8
                if prev_ctx is not None:
                    jprev, pts_prev, onT_prev = prev_ctx
                    onT = norm_pool.tile([P, N], F16, tag="on", name=f"onT{jprev}")
                else:
                    jprev, pts_prev, onT = None, None, None

                def filler(a):
                    if j == 0:
                        # prologue fillers: v^T chunks + next-pair q/k
                        if a == 0:
                            emit_C(2), emit_C(3)
                        elif a == 1:
                            emit_C(4), emit_C(5)
                        elif a == 2:
                            emit_B_half(1, 0)
                        elif a == 3:
                            emit_B_half(1, 1)
                        elif a == 4:
                            emit_C(6), emit_C(7)
                        elif a == 5:
                            emit_B_half(5, 0)
                        elif a == 6:
                            emit_B_half(5, 1)
                        return
                    if a == 0:
                        emit_O_chunk(jprev, pts_prev, onT, 0)
                        emit_O_chunk(jprev, pts_prev, onT, 1)
                    elif a == 1:
                        emit_O_chunk(jprev, pts_prev, onT, 2)
                        emit_O_chunk(jprev, pts_prev, onT, 3)
                    elif a == 2 and j < 3:
                        emit_B_half(j + 1, 0)
                    elif a == 3 and j < 3:
                        emit_B_half(j + 1, 1)
                    elif a == 4:
                        emit_O_chunk(jprev, pts_prev, onT, 4)
                        emit_O_chunk(jprev, pts_prev, onT, 5)
                    elif a == 5 and j < 3:
                        emit_B_half(j + 5, 0)
                    elif a == 6 and j < 3:
                        emit_B_half(j + 5, 1)
                    elif a == 7:
                        emit_O_chunk(jprev, pts_prev, onT, 6)
                        emit_O_chunk(jprev, pts_prev, onT, 7)
                        emit_transposes(jprev, onT)

                qt = qk_sb[j]
                kt = qk_sb[4 + j]
                for a in range(8):
                    s_ps0 = ps.tile([P, N], F32, tag="s0", name=f"s0_{j}_{a}")
                    s_ps1 = ps.tile([P, N], F32, tag="s1", name=f"s1_{j}_{a}")
                    for ih in range(2):
                        sl = slice(ih * 512, (ih + 1) * 512)
                        nc.tensor.matmul(
                            s_ps0[:, sl],
                            kt[0:64, a * P : (a + 1) * P],
                            qt[0:64, sl],
                            start=True,
                            stop=True,
                        )
                        nc.tensor.matmul(
                            s_ps1[:, sl],
                            kt[64:128, a * P : (a + 1) * P],
                            qt[64:128, sl],
                            start=True,
                            stop=True,
                        )
                    ptA[a] = pt_pool.tile([P, N], F16, tag=f"ptA{a}", name=f"ptA{j}_{a}")
                    ptB[a] = pt_pool.tile([P, N], F16, tag=f"ptB{a}", name=f"ptB{j}_{a}")
                    nc.scalar.activation(
                        ptA[a][:], s_ps0[:], EXP, scale=SCALE, bias=eshift_sb[:]
                    )
                    nc.scalar.activation(
                        ptB[a][:], s_ps1[:], EXP, scale=SCALE, bias=eshift_sb[:]
                    )
                    filler(a)
                return (j, (ptA, ptB), None)

            # ---- prologue: first q/k chunks + first v chunks ------------
            emit_B_half(0, 0, tag="s0")
            emit_B_half(0, 1, tag="s0")
            emit_B_half(4, 0, tag="s1")
            emit_B_half(4, 1, tag="s1")
            emit_C(0)
            emit_C(1)

            prev_ctx = None
            for j in range(4):
                new_ctx = emit_region(j, prev_ctx)
                if prev_ctx is not None:
                    prev_ctx = (prev_ctx[0], prev_ctx[1], None)
                prev_ctx = new_ctx

            # ---- tail: O/normalize/transpose for pair 3 -----------------
            j3, pts3, _ = prev_ctx
            onT3 = norm_pool.tile([P, N], F16, tag="on", name="onT3")
            for c in range(8):
                emit_O_chunk(3, pts3, onT3, c)
            emit_transposes(3, onT3)

            # ---- phase E: output projection + residual(+bias) -----------
            # wave A: m=0,1,2 on tags s0,s1,tp; wave B: m=3 on s0.
            dma_engines = (nc.sync, nc.scalar, nc.sync, nc.scalar)

            def emit_E_m(m, tag):
                psum = ps.tile([P, N], F32, tag=tag, name=f"psE{m}")
                for e in range(4):
                    for ih in range(2):
                        sl = slice(ih * 512, (ih + 1) * 512)
                        nc.tensor.matmul(
                            psum[:, sl],
                            woutT[e][:, m * P : (m + 1) * P],
                            osb[e][:, sl],
                            start=(e == 0),
                            stop=(e == 3),
                        )
                out_sb = misc_pool.tile([P, N], F32, tag="outsb", name=f"out_sb{m}")
                for ih in range(2):
                    sl = slice(ih * 512, (ih + 1) * 512)
                    nc.vector.tensor_add(out_sb[:, sl], psum[:, sl], xb[m][:, sl])
                    dma_engines[(2 * m + ih) % 4].dma_start(
                        out_d[m * P : (m + 1) * P, sl], out_sb[:, sl]
                    )

            emit_E_m(0, "s0")
            emit_E_m(1, "s1")
            emit_E_m(2, "tp")
            emit_E_m(3, "s1")

    nc.compile()
    return nc


_CACHE = {}


def _get_nc(reps=1):
    key = reps
    if key not in _CACHE:
        _CACHE[key] = _build(reps=reps)
    return _CACHE[key]


def kernel(x, w_in, w_out, b_out, heads):
    x = np.asarray(x)
    w_in = np.asarray(w_in)
    w_out = np.asarray(w_out)
    b_out = np.asarray(b_out)
    B = x.shape[0]
    assert int(heads) == HEADS, f"kernel compiled for heads=8, got {heads}"
    assert x.shape == (B, C, 32, 32) and B == N_CORES

    xf = np.ascontiguousarray(x.reshape(B, C, N), dtype=np.float32)
    wqkT = np.ascontiguousarray(w_in[: 2 * E].T, dtype=np.float32)
    wvT = np.ascontiguousarray(w_in[2 * E :].T, dtype=np.float32)
    woutT = np.ascontiguousarray(w_out.T, dtype=np.float32)
    bias = np.ascontiguousarray(b_out.reshape(4, P).T, dtype=np.float32)
    in_maps = [
        {"x": xf[b], "wqkT": wqkT, "wvT": wvT, "woutT": woutT, "bias": bias}
        for b in range(B)
    ]

    nc = _get_nc()
    res = run_bass_kernel_spmd(nc, in_maps, core_ids=list(range(N_CORES)))
    out = np.stack([r["out"] for r in res.results])
    return out.reshape(B, C, 32, 32).astype(x.dtype, copy=False)


# revision 3
# speedup vs baseline: 1.0638x; 1.0638x over previous
"""Trainium2 Bass kernel v2 for the per-batch attention block.

Reference math (per batch b, with C=E=512, H=W=32 -> N=1024, heads=8, d=64):
    qkv = w_in @ x_flat                      # [3E, N]
    S_h = q_h^T k_h * heads**-0.5            # [N, N] per head
    P_h = softmax(S_h, axis=-1)
    o_h = v_h @ P_h^T                        # [d, N]
    out = w_out @ concat(o_h) + b_out + x_flat

Data-parallel over batch across 8 NeuronCores (one batch element/core).

v2 layout changes vs v1:
  - O matmuls produce o^T (positions on partitions): per (head, i-chunk)
    psum [128, 65] accumulated over 8 k-chunks; col 0 is the softmax
    denominator (ones-column trick), so normalization is a per-partition
    reciprocal + tensor_scalar fused into the PSUM evacuation. This kills
    the DRAM-bounce broadcast / gpsimd multiply of v1 and halves the O
    matmul cycles (full 128-row PE utilization).
  - o^T -> o via PE transposes ([128,128] blocks against an identity),
    evacuated to SBUF for the output projection.
  - bias folded into the residual input (xb = x + b) once outside the
    reps loop; no K=1 bias matmuls.
  - software-pipelined schedule: region j emits S/exp(pair j) on PE/ACT
    while interleaving O+normalize(pair j-1), q/k projections for pair
    j+1, and transposes, keeping ACT (the exp wall, ~66us) saturated.
"""

import sys

if "/opt/trn_rl_repo" not in sys.path:
    sys.path.insert(0, "/opt/trn_rl_repo")

from contextlib import ExitStack, nullcontext

import numpy as np

import concourse.bass as bass
import concourse.tile as tile
from concourse import bacc, mybir
from concourse.bass_utils import run_bass_kernel_spmd
from concourse.masks import make_identity

F32 = mybir.dt.float32
F32R = mybir.dt.float32r
F16 = mybir.dt.float16
I32 = mybir.dt.int32
# Schraudolph exp-as-bitcast constants (for the DVE-offloaded tiles)
LOG2E = 1.4426950408889634
DVE_EXP_SLOTS = (2, 5)
ESHIFT = -10.0  # exp bias shift: keeps P = exp(S*scale-10) within fp16 range
EXP = mybir.ActivationFunctionType.Exp

C = 512
N = 1024
E = 512
HEADS = 8
D = 64
NH = D + 1  # ones column + 64 v-channels per head
SCALE = float(HEADS) ** -0.5
SA = float(2**23) * SCALE * LOG2E
SB = float(2**23) * (ESHIFT * LOG2E + 127.0 - 0.0450466)
P = 128
N_CORES = 8


def _build(n_cores=N_CORES, reps=1):
    nc = bacc.Bacc(
        "TRN2", target_bir_lowering=False, debug=False, num_devices=n_cores
    )
    x_d = nc.dram_tensor("x", [C, N], F32R, kind="ExternalInput").ap()
    wqkT_d = nc.dram_tensor("wqkT", [C, 2 * E], F32R, kind="ExternalInput").ap()
    wvT_d = nc.dram_tensor("wvT", [C, E], F32R, kind="ExternalInput").ap()
    woutT_d = nc.dram_tensor("woutT", [E, C], F32R, kind="ExternalInput").ap()
    # bias pre-shaped [128, 4] host-side: col c = b_out[c*128:(c+1)*128]
    bias_d = nc.dram_tensor("bias", [P, 4], F32, kind="ExternalInput").ap()
    out_d = nc.dram_tensor("out", [C, N], F32, kind="ExternalOutput").ap()

    with tile.TileContext(nc) as tc, ExitStack() as ctx:
        consts = ctx.enter_context(tc.tile_pool(name="consts", bufs=1))
        qk_pool = ctx.enter_context(tc.tile_pool(name="qk", bufs=1))
        vt_pool = ctx.enter_context(tc.tile_pool(name="vt", bufs=1))
        osb_pool = ctx.enter_context(tc.tile_pool(name="osb", bufs=1))
        misc_pool = ctx.enter_context(tc.tile_pool(name="misc", bufs=2))

        # ---- load inputs (issue order = first-use order) ----------------
        xf = []
        wqkT = []
        wvT = []
        woutT = []
        for c in range(4):
            tx = consts.tile([P, N], F32R, tag=f"xf{c}", name=f"xf{c}")
            nc.sync.dma_start(tx[:], x_d[c * P : (c + 1) * P, :])
            xf.append(tx)
            tw = consts.tile([P, 2 * E], F32R, tag=f"wqkT{c}", name=f"wqkT{c}")
            nc.scalar.dma_start(tw[:], wqkT_d[c * P : (c + 1) * P, :])
            wqkT.append(tw)
        for c in range(4):
            t = consts.tile([P, E], F32R, tag=f"wvT{c}", name=f"wvT{c}")
            nc.sync.dma_start(t[:], wvT_d[c * P : (c + 1) * P, :])
            wvT.append(t)
        for e in range(4):
            t = consts.tile([P, C], F32R, tag=f"woutT{e}", name=f"woutT{e}")
            nc.scalar.dma_start(t[:], woutT_d[e * P : (e + 1) * P, :])
            woutT.append(t)
        # bias as a [128, 4] column tile: col c = b_out[c*128:(c+1)*128]
        bias_sb = consts.tile([P, 4], F32, tag="bias", name="bias_sb")
        nc.scalar.dma_start(bias_sb[:], bias_d[:])
        eshift_sb = consts.tile([P, 1], F32, tag="eshift", name="eshift_sb")
        nc.vector.memset(eshift_sb[:], ESHIFT)
        ident = consts.tile([P, P], F16, tag="ident", name="ident")
        make_identity(nc, ident[:])
        # residual-with-bias input, computed once
        xb = []
        for c in range(4):
            t = consts.tile([P, N], F32, tag=f"xb{c}", name=f"xb{c}")
            nc.vector.tensor_scalar_add(
                t[:], xf[c][:].bitcast(F32), bias_sb[:, c : c + 1]
            )
            xb.append(t)

        # persistent attention SBUF tiles
        qk_sb = [None] * 8
        vt_sb = []
        for n in range(8):
            t = vt_pool.tile([P, HEADS * NH], F16, tag=f"vt{n}", name=f"vt{n}")
            vt3 = t[:].rearrange("p (h d) -> p h d", h=HEADS)
            nc.vector.memset(vt3[:, :, 0:1], 1.0)  # ones cols, never rewritten
            vt_sb.append(t)
        osb = []
        for j in range(4):
            t = osb_pool.tile([P, N], F32R, tag=f"osb{j}", name=f"osb{j}")
            osb.append(t)

        rep_ctx = (
            tc.For_i(0, reps, 1, hint_engines=(mybir.EngineType.PE,))
            if reps > 1
            else nullcontext()
        )
        with (
            tc.tile_pool(name="ps", bufs=1, space="PSUM") as ps,
            tc.tile_pool(name="pt", bufs=2) as pt_pool,
            tc.tile_pool(name="norm", bufs=2) as norm_pool,
            rep_ctx,
        ):
            # PSUM budget (8 banks): s0,s1 = [128,1024] (2 banks each) for
            # the two heads' S^T tiles; oc0,oc1 = 1 bank each (C psums +
            # o^T chunk accumulators); tp = [128,1024] (2 banks) shared by
            # q/k projection psums, transposes, and an E-phase wave.

            b_psums = {}

            def emit_B_half(m, half, tag="tp"):
                # half 0: c=0,1 (start), half 1: c=2,3 (stop) + evacuation
                if half == 0:
                    b_psums[m] = ps.tile([P, N], F32, tag=tag, name=f"psB{m}")
                psum = b_psums.pop(m) if half == 1 else b_psums[m]
                for c in (0, 1) if half == 0 else (2, 3):
                    for ih in range(2):
                        nc.tensor.matmul(
                            psum[:, ih * 512 : (ih + 1) * 512],
                            wqkT[c][:, m * P : (m + 1) * P],
                            xf[c][:, ih * 512 : (ih + 1) * 512],
                            start=(c == 0),
                            stop=(c == 3),
                        )
                if half == 1:
                    t = qk_pool.tile([P, N], F16, tag=f"qk{m}", name=f"qk{m}")
                    nc.vector.tensor_copy(t[:], psum[:])
                    qk_sb[m] = t

            def emit_C(n):
                psum = ps.tile([P, E], F32, tag=f"oc{n % 2}", name=f"psC{n}")
                for c in range(4):
                    nc.tensor.matmul(
                        psum[:],
                        xf[c][:, n * P : (n + 1) * P],
                        wvT[c][:],
                        start=(c == 0),
                        stop=(c == 3),
                    )
                t3 = vt_sb[n][:].rearrange("p (h d) -> p h d", h=HEADS)
                nc.vector.tensor_copy(
                    t3[:, :, 1:NH], psum[:].rearrange("p (h d) -> p h d", h=HEADS)
                )

            def emit_O_chunk(jprev, pts, onT, c):
                # o^T accumulation for i-chunk c of pair jprev, both heads:
                # psum [128 i, 65] over 8 k-chunks; col 0 = denominator.
                for hslot in range(2):
                    h = 2 * jprev + hslot
                    o_ps = ps.tile(
                        [P, NH], F32, tag=f"oc{(c + hslot) % 2}", name=f"oT{h}_{c}"
                    )
                    for a in range(8):
                        nc.tensor.matmul(
                            o_ps[:],
                            pts[hslot][a][:, c * P : (c + 1) * P],
                            vt_sb[a][:, h * NH : (h + 1) * NH],
                            start=(a == 0),
                            stop=(a == 7),
                        )
                    r = norm_pool.tile([P, 1], F32, tag=f"r{hslot}", name=f"r{h}_{c}")
                    nc.vector.reciprocal(r[:], o_ps[:, 0:1])
                    nc.vector.tensor_scalar_mul(
                        onT[:, c * P + hslot * D : c * P + (hslot + 1) * D],
                        o_ps[:, 1:NH],
                        r[:],
                    )

            def emit_transposes(jprev, onT):
                tp_ps = ps.tile([P, N], F16, tag="tp", name=f"tp{jprev}")
                for c in range(8):
                    nc.tensor.transpose(
                        tp_ps[:, c * P : (c + 1) * P],
                        onT[:, c * P : (c + 1) * P],
                        ident[:],
                    )
                nc.vector.tensor_copy(osb[jprev][:], tp_ps[:])

            def emit_region(j, prev_ctx):
                # S/exp for pair j; filler work interleaved per a-step.
                ptA = [None] * 8
                ptB = [None] * 8
                if prev_ctx is not None:
                    jprev, pts_prev, onT_prev = prev_ctx
                    onT = norm_pool.tile([P, N], F16, tag="on", name=f"onT{jprev}")
                else:
                    jprev, pts_prev, onT = None, None, None

                def filler(a):
                    if j == 0:
                        # prologue fillers: v^T chunks + next-pair q/k
                        if a == 0:
                            emit_C(2), emit_C(3)
                        elif a == 1:
                            emit_C(4), emit_C(5)
                        elif a == 2:
                            emit_B_half(1, 0)
                        elif a == 3:
                            emit_B_half(1, 1)
                        elif a == 4:
                            emit_C(6), emit_C(7)
                        elif a == 5:
                            emit_B_half(5, 0)
                        elif a == 6:
                            emit_B_half(5, 1)
                        return
                    if a == 0:
                        emit_O_chunk(jprev, pts_prev, onT, 0)
                        emit_O_chunk(jprev, pts_prev, onT, 1)
                    elif a == 1:
                        emit_O_chunk(jprev, pts_prev, onT, 2)
                        emit_O_chunk(jprev, pts_prev, onT, 3)
                    elif a == 2 and j < 3:
                        emit_B_half(j + 1, 0)
                    elif a == 3 and j < 3:
                        emit_B_half(j + 1, 1)
                    elif a == 4:
                        emit_O_chunk(jprev, pts_prev, onT, 4)
                        emit_O_chunk(jprev, pts_prev, onT, 5)
                    elif a == 5 and j < 3:
                        emit_B_half(j + 5, 0)
                    elif a == 6 and j < 3:
                        emit_B_half(j + 5, 1)
                    elif a == 7:
                        emit_O_chunk(jprev, pts_prev, onT, 6)
                        emit_O_chunk(jprev, pts_prev, onT, 7)
                        emit_transposes(jprev, onT)

                qt = qk_sb[j]
                kt = qk_sb[4 + j]
                for a in range(8):
                    s_ps0 = ps.tile([P, N], F32, tag="s0", name=f"s0_{j}_{a}")
                    s_ps1 = ps.tile([P, N], F32, tag="s1", name=f"s1_{j}_{a}")
                    for ih in range(2):
                        sl = slice(ih * 512, (ih + 1) * 512)
                        nc.tensor.matmul(
                            s_ps0[:, sl],
                            kt[0:64, a * P : (a + 1) * P],
                            qt[0:64, sl],
                            start=True,
                            stop=True,
                        )
                        nc.tensor.matmul(
                            s_ps1[:, sl],
                            kt[64:128, a * P : (a + 1) * P],
                            qt[64:128, sl],
                            start=True,
                            stop=True,
                        )
                    ptA[a] = pt_pool.tile([P, N], F16, tag=f"ptA{a}", name=f"ptA{j}_{a}")
                    ptB[a] = pt_pool.tile([P, N], F16, tag=f"ptB{a}", name=f"ptB{j}_{a}")
                    nc.scalar.activation(
                        ptA[a][:], s_ps0[:], EXP, scale=SCALE, bias=eshift_sb[:]
                    )
                    if a in DVE_EXP_SLOTS:
                        # Schraudolph: exp(s*SCALE+ESHIFT) ~= bitcast_f32(
                        # int32(SA*s + SB)) computed on the DVE, offloading
                        # the ACT engine (the exp wall).
                        it32 = pt_pool.tile(
                            [P, N], I32, tag=f"se{a}", name=f"se{j}_{a}"
                        )
                        nc.vector.tensor_scalar(
                            it32[:], s_ps1[:], SA, SB,
                            mybir.AluOpType.mult, mybir.AluOpType.add,
                        )
                        nc.vector.tensor_copy(ptB[a][:], it32[:].bitcast(F32))
                    else:
                        nc.scalar.activation(
                            ptB[a][:], s_ps1[:], EXP, scale=SCALE, bias=eshift_sb[:]
                        )
                    filler(a)
                return (j, (ptA, ptB), None)

            # ---- prologue: first q/k chunks + first v chunks ------------
            emit_B_half(0, 0, tag="s0")
            emit_B_half(0, 1, tag="s0")
            emit_B_half(4, 0, tag="s1")
            emit_B_half(4, 1, tag="s1")
            emit_C(0)
            emit_C(1)

            prev_ctx = None
            for j in range(4):
                new_ctx = emit_region(j, prev_ctx)
                if prev_ctx is not None:
                    prev_ctx = (prev_ctx[0], prev_ctx[1], None)
                prev_ctx = new_ctx

            # ---- tail: O/normalize/transpose for pair 3 -----------------
            j3, pts3, _ = prev_ctx
            onT3 = norm_pool.tile([P, N], F16, tag="on", name="onT3")
            for c in range(8):
                emit_O_chunk(3, pts3, onT3, c)
            emit_transposes(3, onT3)

            # ---- phase E: output projection + residual(+bias) -----------
            # wave A: m=0,1,2 on tags s0,s1,tp; wave B: m=3 on s0.
            dma_engines = (nc.sync, nc.scalar, nc.sync, nc.scalar)

            def emit_E_m(m, tag):
                psum = ps.tile([P, N], F32, tag=tag, name=f"psE{m}")
                for e in range(4):
                    for ih in range(2):
                        sl = slice(ih * 512, (ih + 1) * 512)
                        nc.tensor.matmul(
                            psum[:, sl],
                            woutT[e][:, m * P : (m + 1) * P],
                            osb[e][:, sl],
                            start=(e == 0),
                            stop=(e == 3),
                        )
                out_sb = misc_pool.tile([P, N], F32, tag="outsb", name=f"out_sb{m}")
                for ih in range(2):
                    sl = slice(ih * 512, (ih + 1) * 512)
                    nc.vector.tensor_add(out_sb[:, sl], psum[:, sl], xb[m][:, sl])
                    dma_engines[(2 * m + ih) % 4].dma_start(
                        out_d[m * P : (m + 1) * P, sl], out_sb[:, sl]
                    )

            emit_E_m(0, "s0")
            emit_E_m(1, "s1")
            emit_E_m(2, "tp")
            emit_E_m(3, "s1")

    nc.compile()
    return nc


_CACHE = {}


def _get_nc(reps=1):
    key = reps
    if key not in _CACHE:
        _CACHE[key] = _build(reps=reps)
    return _CACHE[key]


def kernel(x, w_in, w_out, b_out, heads):
    x = np.asarray(x)
    w_in = np.asarray(w_in)
    w_out = np.asarray(w_out)
    b_out = np.asarray(b_out)
    B = x.shape[0]
    assert int(heads) == HEADS, f"kernel compiled for heads=8, got {heads}"
    assert x.shape == (B, C, 32, 32) and B == N_CORES

    xf = np.ascontiguousarray(x.reshape(B, C, N), dtype=np.float32)
    wqkT = np.ascontiguousarray(w_in[: 2 * E].T, dtype=np.float32)
    wvT = np.ascontiguousarray(w_in[2 * E :].T, dtype=np.float32)
    woutT = np.ascontiguousarray(w_out.T, dtype=np.float32)
    bias = np.ascontiguousarray(b_out.reshape(4, P).T, dtype=np.float32)
    in_maps = [
        {"x": xf[b], "wqkT": wqkT, "wvT": wvT, "woutT": woutT, "bias": bias}
        for b in range(B)
    ]

    nc = _get_nc()
    res = run_bass_kernel_spmd(nc, in_maps, core_ids=list(range(N_CORES)))
    out = np.stack([r["out"] for r in res.results])
    return out.reshape(B, C, 32, 32).astype(x.dtype, copy=False)
